# revision 1
# baseline (speedup 1.0000x reference)
"""Trainium2 Bass kernel for nn_AttnDBGNNLayer (8-core SPMD).

kernel(**inputs) takes the FULL inputs (as produced by setup_inputs) and
returns the FULL output (new_A, new_B), distributing across 8 NeuronCores.

Design:
- q-rows of both attentions sharded 8-way (1024 rows/core); K/V computed
  replicated from a feature-major x0^T; A and B q-groups interleaved so the
  TensorEngine always has independent work; single-pass unnormalized softmax
  (scores are tiny; no max subtraction); out-projection folded into V
  (Wvo = Wout @ Wv); softmax row-sum accumulated on DVE+GpSimd, finished
  with a ones-matmul; normalize via PE-transpose + per-partition scale.
- per-graph tables hold h @ wl^T (lin_l folded); the three tables are
  concatenated and AllGathered in two row-halves so the first collective
  hides under the second attention q-group.
- message aggregation as dense count-matrix matmuls: out^T += tab_g^T @ C_g
  with C_g the per-core [8192 src, 1024 dst] edge-count matrix in fp8
  (counts are small ints -> exact); C rows are host-permuted to match the
  AllGather row order and host-swizzled for contiguous streaming.
  lin_r / biases / degree corrections fold in as K=1 matmuls into the same
  PSUM accumulation group.
- outputs are produced feature-major and transposed on the host.
"""
import sys

if "/opt/trn_rl_repo" not in sys.path:
    sys.path.insert(0, "/opt/trn_rl_repo")

import numpy as np
import ml_dtypes

import concourse.bacc as bacc
import concourse.tile as tile
import concourse.mybir as mybir
from concourse import bass_utils

BF16 = ml_dtypes.bfloat16
FP8 = ml_dtypes.float8_e4m3

N = 8192
D = 128
NCORES = 8
R = N // NCORES       # 1024 rows per core
QG = 512              # q-group width
KB = N // 128         # 64 k-blocks
NBLK = R // 128       # 8 dst blocks per core
SCALE = 1.0 / np.sqrt(np.float32(D))

F32 = mybir.dt.float32
BF = mybir.dt.bfloat16
F8 = mybir.dt.float8e4

G = ("AB", "BA", "AA")
GI = {g: i for i, g in enumerate(G)}
SRC_T = {"AB": "A", "BA": "B", "AA": "A"}
GRAPHS_OF = {"A": ("BA", "AA"), "B": ("AB",)}
TABLES_OF = {"A": ("AB", "AA"), "B": ("BA",)}

# bf16 weight blob layout: [128,128] slices
WB_ORDER = ["wqT_A", "wkT_A", "wvoT_A", "wqT_B", "wkT_B", "wvoT_B",
            "wlT_AB", "wlT_BA", "wlT_AA", "wrT_A", "wrT_B"]
# f32 col blob: [128, 4]
CB_ORDER = ["bqs_A", "bk_A", "bqs_B", "bk_B"]
# f32 row blob: [1, 128*5 + 1024*3]
RB_ORDER = ["c0_A", "c0_B", "c1_AB", "c1_BA", "c1_AA"]

_PROG_CACHE = {}


def build_program(dbg=False, stage=3):
    nc = bacc.Bacc("TRN2", target_bir_lowering=False, debug=False,
                   num_devices=NCORES)

    x0t = {t: nc.dram_tensor(f"x0t_{t}", [128, N], BF, kind="ExternalInput")
           for t in "AB"}
    x0q = {t: nc.dram_tensor(f"x0q_{t}", [128, R], BF, kind="ExternalInput")
           for t in "AB"}
    wblob = nc.dram_tensor("wblob", [128, 128 * len(WB_ORDER)], BF,
                           kind="ExternalInput")
    cblob = nc.dram_tensor("cblob", [128, len(CB_ORDER)], F32,
                           kind="ExternalInput")
    rblob = nc.dram_tensor("rblob", [1, 128 * 5 + R * 3], F32,
                           kind="ExternalInput")
    ct = {g: nc.dram_tensor(f"ct_{g}", [1024, 8 * R], F8,
                            kind="ExternalInput") for g in G}
    out_d = {t: nc.dram_tensor(f"out_{t}", [128, R], F32,
                               kind="ExternalOutput") for t in "AB"}
    dbg_d = {}
    if dbg:
        for t in "AB":
            dbg_d[f"ht_{t}"] = nc.dram_tensor(f"dbg_ht_{t}", [128, R], BF,
                                              kind="ExternalOutput")
        dbg_d["tab"] = nc.dram_tensor("dbg_tab", [N, 384], BF,
                                      kind="ExternalOutput")

    tab_loc = nc.dram_tensor("tab_loc", [R, 384], BF)
    tab = nc.dram_tensor("tab", [N, 384], BF, addr_space="Shared")

    with tile.TileContext(nc) as tc:
        with (
            tc.tile_pool(name="const", bufs=1) as cp,
            tc.tile_pool(name="big", bufs=1) as bp,
            tc.tile_pool(name="pt", bufs=4) as ptp,
            tc.tile_pool(name="ctp", bufs=7) as ctp,
            tc.tile_pool(name="ps_s", bufs=2, space="PSUM") as ps_s,
            tc.tile_pool(name="ps_u", bufs=2, space="PSUM") as ps_u,
            tc.tile_pool(name="ps_sm", bufs=2, space="PSUM") as ps_sm,
        ):
            # ---------------- inputs: x0 first, then const blobs
            x0_s = {}
            x0q_s = {}
            for t in "AB":
                x0_s[t] = bp.tile([128, N], BF, tag=f"x0t_{t}",
                                  name=f"x0_{t}")
                nc.sync.dma_start(out=x0_s[t][:], in_=x0t[t][:])
                x0q_s[t] = bp.tile([128, R], BF, tag=f"x0q_{t}",
                                   name=f"x0q_{t}")
                nc.sync.dma_start(out=x0q_s[t][:], in_=x0q[t][:])

            wb = cp.tile([128, 128 * len(WB_ORDER)], BF, tag="wb")
            nc.sync.dma_start(out=wb[:], in_=wblob[:])
            W = {k: wb[:, i * 128:(i + 1) * 128]
                 for i, k in enumerate(WB_ORDER)}
            cb = cp.tile([128, len(CB_ORDER)], F32, tag="cb")
            nc.sync.dma_start(out=cb[:], in_=cblob[:])
            C = {k: cb[:, i:i + 1] for i, k in enumerate(CB_ORDER)}
            rb = cp.tile([1, 128 * 5 + R * 3], F32, tag="rb")
            nc.sync.dma_start(out=rb[:], in_=rblob[:])
            RW = {k: rb[:, i * 128:(i + 1) * 128]
                  for i, k in enumerate(RB_ORDER)}
            DEG = {g: rb[:, 640 + GI[g] * R: 640 + (GI[g] + 1) * R] for g in G}

            ident = cp.tile([128, 128], BF, tag="ident")
            from concourse.masks import make_identity
            make_identity(nc, ident[:])
            ones_col = cp.tile([128, 1], BF, tag="ones_col")
            nc.vector.memset(ones_col[:], 1.0)
            ones_row = cp.tile([1, 512], F32, tag="ones_row")
            nc.vector.memset(ones_row[:], 1.0)

            ht = {t: bp.tile([128, R], BF, tag=f"ht_{t}", name=f"ht_{t}")
                  for t in "AB"}

            # ---------------- QKV for both types
            kt = {}
            vt = {}
            qt = {}
            for t in "AB":
                kt[t] = bp.tile([128, N], BF, tag=f"kt_{t}", name=f"kt_{t}")
                vt[t] = bp.tile([128, N], BF, tag=f"vt_{t}", name=f"vt_{t}")
                qt[t] = bp.tile([128, R], BF, tag=f"qt_{t}", name=f"qt_{t}")
                for j in range(N // 512):
                    ps = ps_s.tile([128, 1024], F32, tag="sc")
                    nc.tensor.matmul(ps[:, :512], lhsT=W[f"wkT_{t}"],
                                     rhs=x0_s[t][:, j * 512:(j + 1) * 512],
                                     start=True, stop=True)
                    nc.vector.tensor_scalar_add(
                        kt[t][:, j * 512:(j + 1) * 512], ps[:, :512],
                        C[f"bk_{t}"])
                for j in range(R // 512):
                    ps = ps_s.tile([128, 1024], F32, tag="sc")
                    nc.tensor.matmul(ps[:, :512], lhsT=W[f"wqT_{t}"],
                                     rhs=x0q_s[t][:, j * 512:(j + 1) * 512],
                                     start=True, stop=True)
                    nc.vector.tensor_scalar(
                        qt[t][:, j * 512:(j + 1) * 512], ps[:, :512],
                        float(SCALE), C[f"bqs_{t}"],
                        op0=mybir.AluOpType.mult, op1=mybir.AluOpType.add)
                for vg in range(KB // 4):
                    ps = ps_u.tile([128, 512], F32, tag="ut")
                    for i in range(4):
                        nb = vg * 4 + i
                        nc.tensor.matmul(ps[:, i * 128:(i + 1) * 128],
                                         lhsT=x0_s[t][:, nb * 128:(nb + 1) * 128],
                                         rhs=W[f"wvoT_{t}"],
                                         start=True, stop=True)
                    nc.vector.tensor_copy(vt[t][:, vg * 512:(vg + 1) * 512],
                                          ps[:])

            # ---------------- attention, A/B interleaved; tables per half
            for qg in range(R // QG):
                q_sl = slice(qg * QG, (qg + 1) * QG)
                ut_ps = {}
                racc0 = {}
                racc1 = {}
                for t in "AB":
                    ut_ps[t] = ps_u.tile([128, QG], F32, tag="ut",
                                         name=f"utps_{t}_{qg}")
                    racc0[t] = bp.tile([128, 2 * QG], BF, tag=f"racc0_{t}",
                                       name=f"racc0_{t}_{qg}")
                    racc1[t] = bp.tile([128, 2 * QG], BF, tag=f"racc1_{t}",
                                       name=f"racc1_{t}_{qg}")
                    nc.vector.memset(racc0[t][:], 0.0)
                    nc.gpsimd.memset(racc1[t][:], 0.0)
                for pr in range(KB // 2):
                    kb0 = 2 * pr
                    for t in "AB":
                        sc = ps_s.tile([128, 1024], F32, tag="sc",
                                       name=f"sc_{t}_{pr}")
                        nc.tensor.matmul(sc[:, :512],
                                         lhsT=kt[t][:, kb0 * 128:(kb0 + 1) * 128],
                                         rhs=qt[t][:, q_sl],
                                         start=True, stop=True)
                        nc.tensor.matmul(sc[:, 512:],
                                         lhsT=kt[t][:, (kb0 + 1) * 128:(kb0 + 2) * 128],
                                         rhs=qt[t][:, q_sl],
                                         start=True, stop=True)
                        pt = ptp.tile([128, 1024], BF, tag="pt",
                                      name=f"pt_{t}_{pr}")
                        nc.scalar.activation(pt[:], sc[:],
                                             mybir.ActivationFunctionType.Exp)
                        nc.tensor.matmul(ut_ps[t][:],
                                         lhsT=vt[t][:, kb0 * 128:(kb0 + 1) * 128],
                                         rhs=pt[:, :512],
                                         start=(pr == 0), stop=False)
                        nc.tensor.matmul(ut_ps[t][:],
                                         lhsT=vt[t][:, (kb0 + 1) * 128:(kb0 + 2) * 128],
                                         rhs=pt[:, 512:],
                                         start=False, stop=(pr == KB // 2 - 1))
                        if pr % 4 != 3:
                            nc.vector.tensor_add(racc0[t][:], racc0[t][:],
                                                 pt[:])
                        else:
                            nc.gpsimd.tensor_tensor(racc1[t][:], racc1[t][:],
                                                    pt[:],
                                                    op=mybir.AluOpType.add)

                # normalize + both orientations of h
                for t in "AB":
                    ut_sb = bp.tile([128, QG], BF, tag=f"ut_sb_{t}",
                                    name=f"ut_sb_{t}_{qg}")
                    nc.vector.tensor_copy(ut_sb[:], ut_ps[t][:])
                    for sub in range(QG // 128):
                        s_sl = slice(sub * 128, (sub + 1) * 128)
                        rp = ps_sm.tile([128, 512], F32, tag="sm", name="rp")
                        nc.tensor.matmul(rp[:, :1], lhsT=racc0[t][:, s_sl],
                                         rhs=ones_col[:], start=True,
                                         stop=False)
                        nc.tensor.matmul(rp[:, :1],
                                         lhsT=racc0[t][:, 512 + sub * 128:
                                                      512 + (sub + 1) * 128],
                                         rhs=ones_col[:], start=False,
                                         stop=False)
                        nc.tensor.matmul(rp[:, :1], lhsT=racc1[t][:, s_sl],
                                         rhs=ones_col[:], start=False,
                                         stop=False)
                        nc.tensor.matmul(rp[:, :1],
                                         lhsT=racc1[t][:, 512 + sub * 128:
                                                      512 + (sub + 1) * 128],
                                         rhs=ones_col[:], start=False,
                                         stop=True)
                        rinv = bp.tile([128, 1], F32, tag="rinv")
                        nc.vector.reciprocal(rinv[:], rp[:, :1])
                        tp = ps_sm.tile([128, 512], BF, tag="sm", name="tp")
                        nc.tensor.transpose(tp[:, :128], ut_sb[:, s_sl],
                                            ident[:])
                        hn = bp.tile([128, 128], BF, tag="hn")
                        nc.vector.tensor_scalar_mul(hn[:], tp[:, :128],
                                                    rinv[:, :])
                        tp2 = ps_sm.tile([128, 512], BF, tag="sm", name="tp2")
                        nc.tensor.transpose(tp2[:, :128], hn[:], ident[:])
                        nc.vector.tensor_copy(
                            ht[t][:, qg * QG + sub * 128:
                                  qg * QG + (sub + 1) * 128],
                            tp2[:, :128])

                # tables for this half: local rows qg*512..+512, all graphs
                for t in "AB":
                    for g in TABLES_OF[t]:
                        tsb = bp.tile([128, 4 * 128], BF, tag="tsb",
                                      name=f"tsb_{g}_{qg}")
                        for nb in range(4):
                            wblk = qg * 4 + nb
                            ps = ps_sm.tile([128, 512], F32, tag="sm",
                                            name="tps")
                            nc.tensor.matmul(
                                ps[:, :128],
                                lhsT=ht[t][:, wblk * 128:(wblk + 1) * 128],
                                rhs=W[f"wlT_{g}"], start=True, stop=True)
                            nc.vector.tensor_copy(
                                tsb[:, nb * 128:(nb + 1) * 128], ps[:, :128])
                        for nb in range(4):
                            wblk = qg * 4 + nb
                            nc.sync.dma_start(
                                out=tab_loc[wblk * 128:(wblk + 1) * 128,
                                            GI[g] * 128:(GI[g] + 1) * 128],
                                in_=tsb[:, nb * 128:(nb + 1) * 128])
                # half AllGather: rows [qg*512, qg*512+512) of each core
                nc.gpsimd.collective_compute(
                    "AllGather", mybir.AluOpType.bypass,
                    replica_groups=[list(range(NCORES))],
                    ins=[tab_loc[qg * 512:(qg + 1) * 512, :]],
                    outs=[tab[qg * 4096:(qg + 1) * 4096, :]])

            if dbg:
                for t in "AB":
                    nc.sync.dma_start(out=dbg_d[f"ht_{t}"][:], in_=ht[t][:])
                nc.sync.dma_start(out=dbg_d["tab"][:], in_=tab[:])

            # ---------------- phase 2: dense count-matrix aggregation
            # out^T[d, dst] = sum_g tab_g^T @ C_g + wr@ht + c0*1 + c1*deg
            for t in ("BA" if stage >= 2 else ""):
                po = []
                for h in range(2):
                    po_t = ps_sm.tile([128, 512], F32, tag="sm",
                                      name=f"po_{t}_{h}")
                    po.append(po_t)
                first = [True, True]
                for g in GRAPHS_OF[t]:
                    tabsb = bp.tile([128, KB * 128], BF,
                                    tag=f"x0t_{'A' if GI[g] % 2 == 0 else 'B'}",
                                    name=f"tabsb_{g}")
                    for half in range(2):
                        nc.sync.dma_start(
                            out=tabsb[:, half * 4096:(half + 1) * 4096]
                            .rearrange("s (b d) -> s b d", d=128),
                            in_=tab[half * 4096:(half + 1) * 4096,
                                    GI[g] * 128:(GI[g] + 1) * 128]
                            .rearrange("(b s) d -> s b d", s=128))
                    for scg in range(8):
                        ct_t = ctp.tile([128, 8 * R], F8, tag="ct",
                                        name=f"ct_{g}_{scg}")
                        nc.sync.dma_start(
                            out=ct_t[:],
                            in_=ct[g][scg * 128:(scg + 1) * 128, :])
                        for sb in range(8):
                            lt = tabsb[:, (scg * 8 + sb) * 128:
                                       (scg * 8 + sb + 1) * 128]
                            for h in range(2):
                                nc.tensor.matmul(
                                    po[h][:],
                                    lhsT=lt,
                                    rhs=ct_t[:, sb * R + h * 512:
                                             sb * R + (h + 1) * 512],
                                    start=first[h], stop=False)
                                first[h] = False
                    for h in range(2):
                        nc.tensor.matmul(po[h][:], lhsT=RW[f"c1_{g}"],
                                         rhs=DEG[g][:, h * 512:(h + 1) * 512],
                                         start=False, stop=False)
                for h in range(2):
                    nc.tensor.matmul(po[h][:], lhsT=W[f"wrT_{t}"],
                                     rhs=ht[t][:, h * 512:(h + 1) * 512],
                                     start=False, stop=False)
                    nc.tensor.matmul(po[h][:], lhsT=RW[f"c0_{t}"],
                                     rhs=ones_row[:], start=False, stop=True)
                    osb = bp.tile([128, 512], F32, tag="osb",
                                  name=f"osb_{t}_{h}")
                    nc.vector.tensor_copy(osb[:], po[h][:])
                    nc.sync.dma_start(out=out_d[t][:, h * 512:(h + 1) * 512],
                                      in_=osb[:])

    nc.compile()
    return nc


# ---------------------------------------------------------------- host prep

def _row_perm():
    """node id -> table row under the half-AllGather layout."""
    n = np.arange(N)
    c = n >> 10
    w = n & 1023
    return (w >> 9) * 4096 + c * 512 + (w & 511)


def _prep(inputs, dbg=False):
    ins = {k: np.asarray(v) for k, v in inputs.items()}

    def bf(x):
        return np.ascontiguousarray(np.asarray(x, np.float32)).astype(BF16)

    com = {}
    for t in "AB":
        iw = ins[f"inW_{t}"].astype(np.float32)
        ib = ins[f"inB_{t}"].astype(np.float32)
        ow = ins[f"outW_{t}"].astype(np.float32)
        ob = ins[f"outB_{t}"].astype(np.float32)
        com[f"wqT_{t}"] = iw[0:128].T
        com[f"wkT_{t}"] = iw[128:256].T
        com[f"wvoT_{t}"] = (ow @ iw[256:384]).T
        com[f"bqs_{t}"] = ib[0:128] * SCALE
        com[f"bk_{t}"] = ib[128:256]
        com[f"bout_eff_{t}"] = ow @ ib[256:384] + ob
    for g in G:
        com[f"wlT_{g}"] = ins[f"wl_{g}"].astype(np.float32).T
        com[f"c1_{g}"] = (ins[f"wl_{g}"].astype(np.float32)
                          @ com[f"bout_eff_{SRC_T[g]}"])
    com["wrT_B"] = ins["wr_AB"].astype(np.float32).T
    com["wrT_A"] = (ins["wr_BA"] + ins["wr_AA"]).astype(np.float32).T
    com["c0_B"] = (ins["bl_AB"].astype(np.float32)
                   + ins["wr_AB"].astype(np.float32) @ com["bout_eff_B"])
    com["c0_A"] = (ins["bl_BA"].astype(np.float32)
                   + ins["bl_AA"].astype(np.float32)
                   + (ins["wr_BA"] + ins["wr_AA"]).astype(np.float32)
                   @ com["bout_eff_A"])

    wblob = bf(np.concatenate([com[k] for k in WB_ORDER], axis=1))
    cblob = np.stack([com[k] for k in CB_ORDER], axis=1).astype(np.float32)

    x0T = {t: np.ascontiguousarray(
        ins[f"x_{t}"][:, 0, :].astype(np.float32).T).astype(BF16)
        for t in "AB"}

    perm = _row_perm()
    cts = {}
    degs = {}
    for g in G:
        src = np.asarray(ins[f"ei_{g}"][0], np.int64)
        dst = np.asarray(ins[f"ei_{g}"][1], np.int64)
        per_core = []
        dgs = []
        for c in range(NCORES):
            sel = (dst >> 10) == c
            s_c = perm[src[sel]]          # permuted table rows
            d_c = dst[sel] - c * R
            cmat = np.zeros((N, R), np.float32)
            np.add.at(cmat, (s_c, d_c), 1.0)
            swz = np.ascontiguousarray(
                cmat.reshape(8, 8, 128, R).transpose(0, 2, 1, 3)
                .reshape(1024, 8 * R))
            per_core.append(swz.astype(FP8))
            dgs.append(np.bincount(d_c, minlength=R).astype(np.float32))
        cts[g] = per_core
        degs[g] = dgs

    in_maps = []
    for c in range(NCORES):
        rblob = np.concatenate(
            [com[k] for k in RB_ORDER] + [degs[g][c] for g in G]
        ).astype(np.float32).reshape(1, -1)
        m = {"wblob": wblob, "cblob": cblob, "rblob": rblob}
        for t in "AB":
            m[f"x0t_{t}"] = x0T[t]
            m[f"x0q_{t}"] = np.ascontiguousarray(x0T[t][:, c * R:(c + 1) * R])
        for g in G:
            m[f"ct_{g}"] = cts[g][c]
        in_maps.append(m)
    return in_maps


def kernel(**inputs):
    in_maps = _prep(inputs)
    if "prog" not in _PROG_CACHE:
        _PROG_CACHE["prog"] = build_program()
    nc = _PROG_CACHE["prog"]
    res = bass_utils.run_bass_kernel_spmd(
        nc, in_maps, core_ids=list(range(NCORES)))
    x_A = np.asarray(inputs["x_A"], np.float32)
    x_B = np.asarray(inputs["x_B"], np.float32)
    new_A = x_A.copy()
    new_B = x_B.copy()
    for c in range(NCORES):
        new_A[c * R:(c + 1) * R, 0, :] = res.results[c]["out_A"].T
        new_B[c * R:(c + 1) * R, 0, :] = res.results[c]["out_B"].T
    return new_A, new_B



# revision 4
# speedup vs baseline: 1.3687x; 1.3687x over previous
"""Trainium2 Bass kernel for nn_AttnDBGNNLayer (8-core SPMD).

kernel(**inputs) takes the FULL inputs (as produced by setup_inputs) and
returns the FULL output (new_A, new_B), distributing across 8 NeuronCores.

Design (v2):
- q-rows of both attentions sharded 8-way (1024 rows/core); K/V computed from
  a feature-major x0^T (fp8 input); single-pass unnormalized softmax (scores
  tiny, softmax invariant to the k-bias so it is dropped entirely);
  out-projection folded into V (Wvo = Wout @ Wv), V stored fp8.
- exp(scores) emitted in fp8; attn@V and the softmax row-sum (ones-lhsT)
  run as fp8 DoubleRow matmuls accumulating in PSUM — no DVE accumulation.
  Row sums land as [1, 512]; reciprocal + gpsimd partition_broadcast gives a
  per-column scale so normalization is one elementwise multiply (no PE
  transposes).
- normalized h^T is AllGathered per row-half (feature-major, 3x less traffic
  than gathering tables); each core reloads all-cores h^T with two large
  contiguous DMAs and computes per-128-src-block tables on chip
  (tab = h @ (128*wl^T), stored fp8).
- message aggregation as dense count-matrix DoubleRow matmuls:
  out^T += tab_pair^T x4 ct_pair with ct the per-core [8192 src, 1024 dst]
  edge-count matrix scaled by 2^-7 (exact in fp8 for counts <= 15), rows
  permuted to the gathered-h column order and swizzled for paired streaming.
  lin_r / biases / degree corrections fold in as K=1/K=128 f32-path matmuls
  into the same PSUM accumulation group.
- outputs produced feature-major and transposed on the host.
"""
import sys

if "/opt/trn_rl_repo" not in sys.path:
    sys.path.insert(0, "/opt/trn_rl_repo")

import numpy as np
import ml_dtypes

import concourse.bacc as bacc
import concourse.tile as tile
import concourse.mybir as mybir
from concourse import bass_utils

BF16 = ml_dtypes.bfloat16
FP8 = ml_dtypes.float8_e4m3

N = 8192
D = 128
NCORES = 8
R = N // NCORES       # 1024 rows per core
QG = 512              # q-group width (one AllGather half)
KB = N // 128         # 64 k-blocks
SCALE = 1.0 / np.sqrt(np.float32(D))
TAB_SCALE = 128.0     # tables stored as tab*128 fp8; counts scaled by 2^-7

F32 = mybir.dt.float32
BF = mybir.dt.bfloat16
F8 = mybir.dt.float8e4
DR = mybir.MatmulPerfMode.DoubleRow

G = ("AB", "BA", "AA")
GI = {g: i for i, g in enumerate(G)}
SRC_T = {"AB": "A", "BA": "B", "AA": "A"}
DST_T = {"AB": "B", "BA": "A", "AA": "A"}

# bf16 weight blob layout: [128,128] slices (wlT_AB|wlT_AA adjacent = wl_cat_A)
WB_ORDER = ["wqT_A", "wkT_A", "wvoT_A", "wqT_B", "wkT_B", "wvoT_B",
            "wlT_AB", "wlT_AA", "wlT_BA", "wrT_A", "wrT_B"]
# f32 col blob: [128, 2]
CB_ORDER = ["bqs_A", "bqs_B"]
# f32 row blob: [1, 128*5 + 1024*3]
RB_ORDER = ["c0_A", "c0_B", "c1_AB", "c1_BA", "c1_AA"]

_PROG_CACHE = {}


def build_program(dbg=False, stage=3):
    nc = bacc.Bacc("TRN2", target_bir_lowering=False, debug=False,
                   num_devices=NCORES)

    x0t = {t: nc.dram_tensor(f"x0t_{t}", [128, N], F8, kind="ExternalInput")
           for t in "AB"}
    x0q = {t: nc.dram_tensor(f"x0q_{t}", [128, R], F8, kind="ExternalInput")
           for t in "AB"}
    wblob = nc.dram_tensor("wblob", [128, 128 * len(WB_ORDER)], BF,
                           kind="ExternalInput")
    cblob = nc.dram_tensor("cblob", [128, len(CB_ORDER)], F32,
                           kind="ExternalInput")
    rblob = nc.dram_tensor("rblob", [1, 128 * 5 + R * 3], F32,
                           kind="ExternalInput")
    ct = {g: nc.dram_tensor(f"ct_{g}", [1024, 8 * R], F8,
                            kind="ExternalInput") for g in G}
    out_d = {t: nc.dram_tensor(f"out_{t}", [128, R], F32,
                               kind="ExternalOutput") for t in "AB"}
    dbg_d = {}
    if dbg:
        for t in "AB":
            dbg_d[f"ht_{t}"] = nc.dram_tensor(f"dbg_ht_{t}", [128, R], BF,
                                              kind="ExternalOutput")
            dbg_d[f"hT_{t}"] = nc.dram_tensor(f"dbg_hT_{t}", [128, N], BF,
                                              kind="ExternalOutput")

    h_loc = {t: nc.dram_tensor(f"h_loc_{t}", [256, QG], BF) for t in "AB"}
    h_sh = {t: nc.dram_tensor(f"h_sh_{t}", [2048, QG], BF,
                              addr_space="Shared") for t in "AB"}

    with tile.TileContext(nc) as tc:
        with (
            tc.tile_pool(name="const", bufs=1) as cp,
            tc.tile_pool(name="big", bufs=1) as bp,
            tc.tile_pool(name="pt", bufs=4) as ptp,
            tc.tile_pool(name="ctp", bufs=7) as ctp,
            tc.tile_pool(name="tabp", bufs=2) as tabp,
            tc.tile_pool(name="ps_big", bufs=2, space="PSUM") as ps_big,
            tc.tile_pool(name="ps_ut", bufs=2, space="PSUM") as ps_ut,
            tc.tile_pool(name="ps_rs", bufs=2, space="PSUM") as ps_rs,
        ):
            # ---------------- inputs
            x0_s = {}
            x0q_s = {}
            for t in "AB":
                x0_s[t] = bp.tile([128, N], F8, tag=f"x0t_{t}",
                                  name=f"x0_{t}")
                nc.sync.dma_start(out=x0_s[t][:], in_=x0t[t][:])
                x0q_s[t] = bp.tile([128, R], F8, tag=f"x0q_{t}",
                                   name=f"x0q_{t}")
                nc.sync.dma_start(out=x0q_s[t][:], in_=x0q[t][:])

            wb = cp.tile([128, 128 * len(WB_ORDER)], BF, tag="wb")
            nc.sync.dma_start(out=wb[:], in_=wblob[:])
            W = {k: wb[:, i * 128:(i + 1) * 128]
                 for i, k in enumerate(WB_ORDER)}
            wl_cat_A = wb[:, 6 * 128:8 * 128]  # [wlT_AB | wlT_AA]
            cb = cp.tile([128, len(CB_ORDER)], F32, tag="cb")
            nc.sync.dma_start(out=cb[:], in_=cblob[:])
            C = {k: cb[:, i:i + 1] for i, k in enumerate(CB_ORDER)}
            rb = cp.tile([1, 128 * 5 + R * 3], F32, tag="rb")
            nc.sync.dma_start(out=rb[:], in_=rblob[:])
            RW = {k: rb[:, i * 128:(i + 1) * 128]
                  for i, k in enumerate(RB_ORDER)}
            DEG = {g: rb[:, 640 + GI[g] * R: 640 + (GI[g] + 1) * R] for g in G}

            # DoubleRow lhsT needs the k-pair dim stride %16 == 0
            ones2 = cp.tile([128, 32], F8, tag="ones2")
            nc.vector.memset(ones2[:], 1.0)
            ones_row = cp.tile([1, 512], F32, tag="ones_row")
            nc.vector.memset(ones_row[:], 1.0)

            ht = {t: bp.tile([128, R], BF, tag=f"ht_{t}", name=f"ht_{t}")
                  for t in "AB"}

            # ---------------- QKV for both types
            kt = {}
            vt = {}
            qt = {}
            copy_i = 0
            for t in "AB":
                kt[t] = bp.tile([128, N], BF, tag=f"kt_{t}", name=f"kt_{t}")
                vt[t] = bp.tile([128, N], F8, tag=f"vt_{t}", name=f"vt_{t}")
                qt[t] = bp.tile([128, R], BF, tag=f"qt_{t}", name=f"qt_{t}")
                ps_q = ps_big.tile([128, 1024], F32, tag="sc",
                                   name=f"psq_{t}")
                for j in range(2):
                    nc.tensor.matmul(ps_q[:, j * 512:(j + 1) * 512],
                                     lhsT=W[f"wqT_{t}"],
                                     rhs=x0q_s[t][:, j * 512:(j + 1) * 512],
                                     start=True, stop=True)
                nc.vector.tensor_scalar(
                    qt[t][:], ps_q[:], float(SCALE), C[f"bqs_{t}"],
                    op0=mybir.AluOpType.mult, op1=mybir.AluOpType.add)
                for j4 in range(8):
                    ps_k = ps_big.tile([128, 1024], F32, tag="sc",
                                       name=f"psk_{t}_{j4}")
                    for j in range(2):
                        nc.tensor.matmul(
                            ps_k[:, j * 512:(j + 1) * 512],
                            lhsT=W[f"wkT_{t}"],
                            rhs=x0_s[t][:, j4 * 1024 + j * 512:
                                        j4 * 1024 + (j + 1) * 512],
                            start=True, stop=True)
                    dst = kt[t][:, j4 * 1024:(j4 + 1) * 1024]
                    if copy_i % 2 == 0:
                        nc.scalar.activation(
                            dst, ps_k[:], mybir.ActivationFunctionType.Copy)
                    else:
                        nc.vector.tensor_copy(dst, ps_k[:])
                    copy_i += 1
                for vg in range(16):
                    ps_v = ps_ut.tile([128, 512], F32, tag="ut",
                                      name=f"psv_{t}_{vg}")
                    for i in range(4):
                        nb = vg * 4 + i
                        nc.tensor.matmul(ps_v[:, i * 128:(i + 1) * 128],
                                         lhsT=x0_s[t][:, nb * 128:(nb + 1) * 128],
                                         rhs=W[f"wvoT_{t}"],
                                         start=True, stop=True)
                    dst = vt[t][:, vg * 512:(vg + 1) * 512]
                    if copy_i % 2 == 0:
                        nc.scalar.activation(
                            dst, ps_v[:], mybir.ActivationFunctionType.Copy)
                    else:
                        nc.vector.tensor_copy(dst, ps_v[:])
                    copy_i += 1

            # ---------------- attention, A/B interleaved; gather per half
            ones2_3 = ones2[:].rearrange("p (j o) -> p j o", j=2)[:, :, 0:1]
            for qg in range(2):
                q_sl = slice(qg * QG, (qg + 1) * QG)
                ut_ps = {}
                rs_ps = {}
                for t in "AB":
                    ut_ps[t] = ps_ut.tile([128, QG], F32, tag="ut",
                                          name=f"utps_{t}_{qg}")
                    rs_ps[t] = ps_rs.tile([1, QG], F32, tag="rs",
                                          name=f"rsps_{t}_{qg}")
                for pr in range(KB // 2):
                    kb0 = 2 * pr
                    for t in "AB":
                        sc = ps_big.tile([128, 1024], F32, tag="sc",
                                         name=f"sc_{t}_{qg}_{pr}")
                        nc.tensor.matmul(sc[:, :512],
                                         lhsT=kt[t][:, kb0 * 128:(kb0 + 1) * 128],
                                         rhs=qt[t][:, q_sl],
                                         start=True, stop=True)
                        nc.tensor.matmul(sc[:, 512:],
                                         lhsT=kt[t][:, (kb0 + 1) * 128:(kb0 + 2) * 128],
                                         rhs=qt[t][:, q_sl],
                                         start=True, stop=True)
                        pt = ptp.tile([128, 1024], F8, tag="pt",
                                      name=f"pt_{t}_{qg}_{pr}")
                        nc.scalar.activation(pt[:], sc[:],
                                             mybir.ActivationFunctionType.Exp)
                        pt3 = pt[:].rearrange("p (j q) -> p j q", j=2)
                        v3 = vt[t][:, kb0 * 128:(kb0 + 2) * 128].rearrange(
                            "p (j f) -> p j f", j=2)
                        nc.tensor.matmul(ut_ps[t][:], lhsT=v3, rhs=pt3,
                                         start=(pr == 0),
                                         stop=(pr == KB // 2 - 1),
                                         perf_mode=DR)
                        nc.tensor.matmul(rs_ps[t][:], lhsT=ones2_3, rhs=pt3,
                                         start=(pr == 0),
                                         stop=(pr == KB // 2 - 1),
                                         perf_mode=DR)

                # normalize: ht = ut * broadcast(1/rowsum)
                for t in "AB":
                    rv = bp.tile([1, QG], F32, tag=f"rv_{t}",
                                 name=f"rv_{t}_{qg}")
                    nc.vector.reciprocal(rv[:], rs_ps[t][:])
                    rbc = bp.tile([128, QG], F32, tag=f"rbc_{t}",
                                  name=f"rbc_{t}_{qg}")
                    nc.gpsimd.partition_broadcast(rbc[:], rv[:])
                    nc.vector.tensor_tensor(ht[t][:, q_sl], ut_ps[t][:],
                                            rbc[:], op=mybir.AluOpType.mult)
                    nc.sync.dma_start(
                        out=h_loc[t][qg * 128:(qg + 1) * 128, :],
                        in_=ht[t][:, q_sl])
                    nc.gpsimd.collective_compute(
                        "AllGather", mybir.AluOpType.bypass,
                        replica_groups=[list(range(NCORES))],
                        ins=[h_loc[t][qg * 128:(qg + 1) * 128, :]],
                        outs=[h_sh[t][qg * 1024:(qg + 1) * 1024, :]])

            if dbg:
                for t in "AB":
                    nc.sync.dma_start(out=dbg_d[f"ht_{t}"][:], in_=ht[t][:])

            # ---------------- phase 2: dense count-matrix aggregation
            if stage >= 2:
                hT = {}
                for t in "AB":
                    hT[t] = bp.tile([128, N], BF, tag=f"hT_{t}",
                                    name=f"hT_{t}")
                    for h in range(2):
                        nc.sync.dma_start(
                            out=hT[t][:, h * 4096:(h + 1) * 4096].rearrange(
                                "f (c w) -> f c w", c=8),
                            in_=h_sh[t][h * 1024:(h + 1) * 1024, :].rearrange(
                                "(c f) w -> f c w", f=128))
                if dbg:
                    for t in "AB":
                        nc.sync.dma_start(out=dbg_d[f"hT_{t}"][:],
                                          in_=hT[t][:])

                po = {}
                for t in "AB":
                    po[t] = ps_big.tile([128, 1024], F32, tag="sc",
                                        name=f"po_{t}")
                    for h in range(2):
                        h_sl = slice(h * 512, (h + 1) * 512)
                        nc.tensor.matmul(po[t][:, h_sl], lhsT=W[f"wrT_{t}"],
                                         rhs=ht[t][:, h_sl],
                                         start=True, stop=False)
                        nc.tensor.matmul(po[t][:, h_sl], lhsT=RW[f"c0_{t}"],
                                         rhs=ones_row[:],
                                         start=False, stop=False)
                for g in G:
                    for h in range(2):
                        nc.tensor.matmul(
                            po[DST_T[g]][:, h * 512:(h + 1) * 512],
                            lhsT=RW[f"c1_{g}"],
                            rhs=DEG[g][:, h * 512:(h + 1) * 512],
                            start=False, stop=False)

                for scg in range(8):
                    ct_s = {}
                    for g in G:
                        ct_s[g] = ctp.tile([128, 8 * R], F8, tag="ct",
                                           name=f"ct_{g}_{scg}")
                        nc.sync.dma_start(
                            out=ct_s[g][:],
                            in_=ct[g][scg * 128:(scg + 1) * 128, :])
                    for sbp in range(4):
                        j0 = scg * 8 + sbp * 2
                        tpA = ps_ut.tile([128, 512], F32, tag="ut",
                                         name=f"tpA_{scg}_{sbp}")
                        tpB = ps_ut.tile([128, 512], F32, tag="ut",
                                         name=f"tpB_{scg}_{sbp}")
                        for j in range(2):
                            blk = slice((j0 + j) * 128, (j0 + j + 1) * 128)
                            nc.tensor.matmul(tpA[:, j * 256:(j + 1) * 256],
                                             lhsT=hT["A"][:, blk],
                                             rhs=wl_cat_A,
                                             start=True, stop=True)
                            nc.tensor.matmul(tpB[:, j * 128:(j + 1) * 128],
                                             lhsT=hT["B"][:, blk],
                                             rhs=W["wlT_BA"],
                                             start=True, stop=True)
                        tabA = tabp.tile([128, 512], F8, tag="tabA",
                                         name=f"tabA_{scg}_{sbp}")
                        nc.scalar.activation(tabA[:], tpA[:],
                                             mybir.ActivationFunctionType.Copy)
                        tabB = tabp.tile([128, 256], F8, tag="tabB",
                                         name=f"tabB_{scg}_{sbp}")
                        nc.vector.tensor_copy(tabB[:], tpB[:, :256])
                        tabA3 = tabA[:].rearrange("p (j f) -> p j f", j=2)
                        tabB3 = tabB[:].rearrange("p (j f) -> p j f", j=2)
                        lhsT_of = {"AB": tabA3[:, :, 0:128],
                                   "AA": tabA3[:, :, 128:256],
                                   "BA": tabB3}
                        last = (scg == 7 and sbp == 3)
                        for g in G:
                            for h in range(2):
                                rhs = ct_s[g][:, sbp * 2048 + h * 1024:
                                              sbp * 2048 + (h + 1) * 1024
                                              ].rearrange(
                                                  "p (j d) -> p j d", j=2)
                                is_stop = last and (
                                    g == ("AB" if DST_T[g] == "B" else "AA"))
                                nc.tensor.matmul(
                                    po[DST_T[g]][:, h * 512:(h + 1) * 512],
                                    lhsT=lhsT_of[g], rhs=rhs,
                                    start=False, stop=is_stop,
                                    perf_mode=DR)

                for t in "AB":
                    for h in range(2):
                        osb = bp.tile([128, 512], F32, tag="osb",
                                      name=f"osb_{t}_{h}")
                        nc.vector.tensor_copy(osb[:],
                                              po[t][:, h * 512:(h + 1) * 512])
                        nc.sync.dma_start(
                            out=out_d[t][:, h * 512:(h + 1) * 512],
                            in_=osb[:])

    nc.compile()
    return nc


# ---------------------------------------------------------------- host prep

def _col_of_src():
    """global node id -> hT_all column (= table row position)."""
    src = np.arange(N)
    c = src >> 10
    rr = src & 1023
    half = rr >> 9
    w = rr & 511
    return half * 4096 + c * 512 + w


def _prep(inputs, dbg=False):
    ins = {k: np.asarray(v) for k, v in inputs.items()}

    def bf(x):
        return np.ascontiguousarray(np.asarray(x, np.float32)).astype(BF16)

    com = {}
    for t in "AB":
        iw = ins[f"inW_{t}"].astype(np.float32)
        ib = ins[f"inB_{t}"].astype(np.float32)
        ow = ins[f"outW_{t}"].astype(np.float32)
        ob = ins[f"outB_{t}"].astype(np.float32)
        com[f"wqT_{t}"] = iw[0:128].T
        com[f"wkT_{t}"] = iw[128:256].T
        com[f"wvoT_{t}"] = (ow @ iw[256:384]).T
        com[f"bqs_{t}"] = ib[0:128] * SCALE
        com[f"bout_eff_{t}"] = ow @ ib[256:384] + ob
    for g in G:
        com[f"wlT_{g}"] = ins[f"wl_{g}"].astype(np.float32).T * TAB_SCALE
        com[f"c1_{g}"] = (ins[f"wl_{g}"].astype(np.float32)
                          @ com[f"bout_eff_{SRC_T[g]}"])
    com["wrT_B"] = ins["wr_AB"].astype(np.float32).T
    com["wrT_A"] = (ins["wr_BA"] + ins["wr_AA"]).astype(np.float32).T
    com["c0_B"] = (ins["bl_AB"].astype(np.float32)
                   + ins["wr_AB"].astype(np.float32) @ com["bout_eff_B"])
    com["c0_A"] = (ins["bl_BA"].astype(np.float32)
                   + ins["bl_AA"].astype(np.float32)
                   + (ins["wr_BA"] + ins["wr_AA"]).astype(np.float32)
                   @ com["bout_eff_A"])

    wblob = bf(np.concatenate([com[k] for k in WB_ORDER], axis=1))
    cblob = np.stack([com[k] for k in CB_ORDER], axis=1).astype(np.float32)

    x0T = {t: np.ascontiguousarray(
        ins[f"x_{t}"][:, 0, :].astype(np.float32).T).astype(FP8)
        for t in "AB"}

    col_of = _col_of_src()
    cts = {}
    degs = {}
    for g in G:
        src = np.asarray(ins[f"ei_{g}"][0], np.int64)
        dst = np.asarray(ins[f"ei_{g}"][1], np.int64)
        per_core = []
        dgs = []
        for c in range(NCORES):
            sel = (dst >> 10) == c
            s_c = col_of[src[sel]]
            d_c = dst[sel] - c * R
            cmat = np.zeros((N, R), np.float32)
            np.add.at(cmat, (s_c, d_c), 1.0)
            cmat *= 1.0 / TAB_SCALE
            swz = np.ascontiguousarray(
                cmat.reshape(8, 4, 2, 128, 2, 512).transpose(0, 3, 1, 4, 2, 5)
                .reshape(1024, 8 * R))
            per_core.append(swz.astype(FP8))
            dgs.append(np.bincount(d_c, minlength=R).astype(np.float32))
        cts[g] = per_core
        degs[g] = dgs

    in_maps = []
    for c in range(NCORES):
        rblob = np.concatenate(
            [com[k] for k in RB_ORDER] + [degs[g][c] for g in G]
        ).astype(np.float32).reshape(1, -1)
        m = {"wblob": wblob, "cblob": cblob, "rblob": rblob}
        for t in "AB":
            m[f"x0t_{t}"] = x0T[t]
            m[f"x0q_{t}"] = np.ascontiguousarray(x0T[t][:, c * R:(c + 1) * R])
        for g in G:
            m[f"ct_{g}"] = cts[g][c]
        in_maps.append(m)
    return in_maps


def kernel(**inputs):
    in_maps = _prep(inputs)
    if "prog" not in _PROG_CACHE:
        _PROG_CACHE["prog"] = build_program()
    nc = _PROG_CACHE["prog"]
    res = bass_utils.run_bass_kernel_spmd(
        nc, in_maps, core_ids=list(range(NCORES)))
    x_A = np.asarray(inputs["x_A"], np.float32)
    x_B = np.asarray(inputs["x_B"], np.float32)
    new_A = x_A.copy()
    new_B = x_B.copy()
    for c in range(NCORES):
        new_A[c * R:(c + 1) * R, 0, :] = res.results[c]["out_A"].T
        new_B[c * R:(c + 1) * R, 0, :] = res.results[c]["out_B"].T
    return new_A, new_B
